# revision 43
# baseline (speedup 1.0000x reference)
"""LocalAttention Bass kernel for Trainium2 (8 NeuronCores).

Problem: B=4 H=8 T=8192 D=64, window=128, look_backward=1, causal.
Sharding: pure (B*H) data parallelism — 32 heads / 8 cores = 4 heads each,
processed as 2 head-pairs so q/k DMAs use all 128 SBUF partitions.

Device algorithm (per head, per 128-token window w):
  S^T[k, q] = K_w' @ Q_w^T      (keys on partitions)
  P = exp(S^T * D^-0.5) * causal01
  O[q, 0:64] = P^T @ V, O[q, 64] = P^T @ 1   (row-sums ride as V's ones col)
Host divides O by the row-sum column and reshapes back.

All SBUF data is bf16 (PSUM stays fp32). Engine balance (measured ns):
  * S matmuls: K=64 per head -> heads A/B at tile_position (0,0)/(64,0)
    stream CONCURRENTLY (window-major emission, head-major PSUM banks).
  * PV matmuls: P blocks [128,128] STATIONARY (128 weight cols -> FWL,
    ~55ns/mm), [V|1] slots [128,65] moving -> out [queries, 65].
  * exp 64 pairs split: ~2/3 on ACT (exact, ~1330/pair) and ~1/3 on DVE
    via Schraudolph bit-trick (~1.8% rms; fused with the causal mask:
    int16(sp*184.665*SCALE + maskB) bitcast as bf16; masked lanes get
    bias 2048 -> tiny positive denormals, NEVER 0/negative-bias which
    would hit the bf16 NaN encodings).
  * ACT-path causal mask: mostly Pool tensor_tensor (~1.4us), some DVE
    tensor_tensor (~0.4us, 2x bf16 mode).
  * output cast PSUM->bf16: one DVE tensor_copy per pair (~0.5us).
  * PV emission lags S/exp by LAG pairs (software pipeline).
  * chunk loads split into two half-tiles, first-needed halves DMA'd
    first, so compute starts ~6us earlier (dependency tracking is
    tile-granular).

PSUM layout per 2-window pair j (windows w=2j, 2j+1), HEAD-major so the
two concurrent head streams hit different banks:
  sp [128, 1024] = [A(w) | A(w+1) | B(w) | B(w+1)], blocks = [T1 | T0next]
  op [128, 512]  = [A_w | A_w1 | B_w | B_w1], 128-col strides, 65 used.

Host-side shard prep (inside kernel(), numpy):
  qTp [2, 128, 8320]  — head-pair Q^T, one zero window appended
  kT  [2, 128, 8192]  — head-pair K^T
  vp  [4, 128, 4225]  — per head: 65 V slots x [V(64) | 1], slot 0 zeroed
  mask01 [128, 128]   — within-window causal 0/1 (k <= q), bf16
  maskB  [128, 256]   — f32 [16249*mask01 + 2048*(1-mask01) | 16249]
Output:
  outT [4, 128, 4160] — outT[h, q%W, w*65+i]: cols 0..63 unnormalized O,
                        col 64 row-sum r
"""

import numpy as np
import ml_dtypes

BF16 = np.dtype(ml_dtypes.bfloat16)

B, H, T, D = 4, 8, 8192, 64
W = 128                     # window size
WIN = T // W                # 64 windows per head
NCORES = 8
BH = B * H                  # 32
BH_PER_CORE = BH // NCORES  # 4
NPAIR = BH_PER_CORE // 2    # 2 head pairs per core
CHUNK_W = 16                # windows per load chunk
HC = CHUNK_W // 2           # half-chunk windows (8)
NCHUNK = WIN // CHUNK_W     # 4
NJ = CHUNK_W // 2           # 2-window pairs per chunk (8)
LAG = 2                     # software-pipeline depth in pairs
SCALE = float(D) ** -0.5
DV = D + 1                  # 65

# Schraudolph bf16-exp constants: bits = round(x*EXP_A + EXP_B)
EXP_A = 128.0 / float(np.log(2.0))   # 184.6650
EXP_B = 16256.0 - 7.0                # bias, C=7 centers the rel error
EXP_MASKED = 2048.0                  # masked-lane bias: tiny positive P

_nc_cache = {}
last_perf = None


def _build_nc(skip=()):
    import concourse.tile as tile
    from concourse import bacc
    from concourse import mybir
    from contextlib import ExitStack

    f32 = mybir.dt.float32
    bf16 = mybir.dt.bfloat16
    i16 = mybir.dt.int16
    Exp = mybir.ActivationFunctionType.Exp
    mult = mybir.AluOpType.mult
    add = mybir.AluOpType.add

    nc = bacc.Bacc()
    qTp = nc.dram_tensor("qTp", [NPAIR, W, (WIN + 1) * W], bf16,
                         kind="ExternalInput")
    kT = nc.dram_tensor("kT", [NPAIR, W, T], bf16, kind="ExternalInput")
    vp = nc.dram_tensor("vp", [BH_PER_CORE, W, (WIN + 1) * DV], bf16,
                        kind="ExternalInput")
    mask = nc.dram_tensor("mask01", [W, W], bf16, kind="ExternalInput")
    maskB = nc.dram_tensor("maskB", [W, 2 * W], f32, kind="ExternalInput")
    outT = nc.dram_tensor("outT", [BH_PER_CORE, W, WIN * DV], bf16,
                          kind="ExternalOutput")

    with tile.TileContext(nc) as tc, ExitStack() as ctx:
        cpool = ctx.enter_context(tc.tile_pool(name="cpool", bufs=1))
        qpool = ctx.enter_context(tc.tile_pool(name="qpool", bufs=2))
        kpool = ctx.enter_context(tc.tile_pool(name="kpool", bufs=2))
        vpool = ctx.enter_context(tc.tile_pool(name="vpool", bufs=4))
        opool = ctx.enter_context(tc.tile_pool(name="opool", bufs=3))
        ppool = ctx.enter_context(tc.tile_pool(name="ppool", bufs=5))
        spsum = ctx.enter_context(tc.tile_pool(name="spsum", bufs=3, space="PSUM"))
        opsum = ctx.enter_context(tc.tile_pool(name="opsum", bufs=2, space="PSUM"))

        mtile = cpool.tile([W, W], bf16)
        nc.sync.dma_start(mtile[:], mask[:])
        mbt = cpool.tile([W, 2 * W], f32)
        nc.sync.dma_start(mbt[:], maskB[:])
        z128 = cpool.tile([W, W], bf16)      # P for the all-masked pad window
        nc.vector.memset(z128[:], 0.0)

        mm = nc.tensor.matmul

        def emit_pv(item):
            """PV + output cast for one 2-window pair (P stationary).
            Emits the chunk's stores after its last pair is cast."""
            pt, pt_prev, vcs, oc, jl, p, c0 = item
            sbase = 0
            op = opsum.tile([W, 4 * W], f32, tag="op")
            for h in range(2):
                vc = vcs[h]
                for i in range(2):
                    wl = 2 * jl + i
                    oco = (2 * h + i) * W
                    s0 = (wl - sbase) * DV       # T0 slot offset in vc
                    # T0 contribution: keys of window wl-1
                    if i == 1:
                        t0 = pt[:, 2 * h * 256 + W:2 * h * 256 + 2 * W]
                    elif pt_prev is not None:
                        t0 = pt_prev[:, (2 * h + 1) * 256 + W:
                                     (2 * h + 1) * 256 + 2 * W]
                    else:
                        t0 = z128[:]
                    mm(op[:, oco:oco + DV], t0, vc[:, s0:s0 + DV],
                       start=True, stop=False)
                    # T1 contribution: keys of window wl
                    mm(op[:, oco:oco + DV],
                       pt[:, (2 * h + i) * 256:(2 * h + i) * 256 + W],
                       vc[:, s0 + DV:s0 + 2 * DV],
                       start=False, stop=True)
            if "ocopy" not in skip:
                # one cast for both heads: [128, (h,i,65)] views
                src = op[:].rearrange("p (h i x) -> p h i x", h=2, x=W)
                src = src[:, :, :, 0:DV]
                dst = oc[:].rearrange("p (h j x) -> p h j x", h=2, x=2 * DV)
                dst = dst[:, :, jl, :].rearrange("p h (i e) -> p h i e", e=DV)
                nc.vector.tensor_copy(dst, src)
            if jl == NJ - 1 and "store" not in skip:
                o0 = (c0 // W) * DV
                for h in range(2):
                    nc.sync.dma_start(
                        outT[2 * p + h, :, o0:o0 + CHUNK_W * DV],
                        oc[:, h * NJ * 2 * DV:(h + 1) * NJ * 2 * DV])

        gp = 0  # global pair counter
        for p in range(NPAIR):
            pt_prev = None
            pending = []
            for c in range(NCHUNK):
                c0 = c * CHUNK_W * W
                qc = qpool.tile([W, (CHUNK_W + 1) * W], bf16, tag="qc")
                kc = kpool.tile([W, CHUNK_W * W], bf16, tag="kc")
                vc0 = vpool.tile([W, (CHUNK_W + 1) * DV], bf16, tag="vc")
                vc1 = vpool.tile([W, (CHUNK_W + 1) * DV], bf16, tag="vc")
                vcs = [vc0, vc1]
                if "loads" not in skip:
                    nc.sync.dma_start(qc[:], qTp[p, :, c0:c0 + (CHUNK_W + 1) * W])
                    nc.sync.dma_start(kc[:], kT[p, :, c0:c0 + CHUNK_W * W])
                    for h in range(2):
                        v0 = c * CHUNK_W * DV
                        nc.sync.dma_start(
                            vcs[h][:],
                            vp[2 * p + h, :, v0:v0 + (CHUNK_W + 1) * DV])
                oc = opool.tile([W, 2 * NJ * 2 * DV], bf16, tag="oc")

                for jl in range(NJ):
                    w0 = 2 * jl
                    qb = 0
                    sp = spsum.tile([W, 4 * 2 * W], f32, tag="sp")
                    if "smm" not in skip:
                        for i in range(2):
                            wl = w0 + i
                            for h in range(2):
                                hb = h * 64
                                mm(sp[:, (2 * h + i) * 256:(2 * h + i + 1) * 256],
                                   kc[hb:hb + 64,
                                      (wl - qb) * W:(wl - qb + 1) * W],
                                   qc[hb:hb + 64,
                                      (wl - qb) * W:(wl - qb + 2) * W],
                                   start=True, stop=True)

                    pt = ppool.tile([W, 4 * 2 * W], bf16, tag="pt")
                    if (gp % 3 == 1 or gp >= 62) and "dve_exp" not in skip:
                        # fused Schraudolph exp + causal mask on DVE
                        pt_i = pt[:].bitcast(i16)
                        out3 = pt_i.rearrange("p (g x) -> p g x", x=2 * W)
                        in3 = sp[:].rearrange("p (g x) -> p g x", x=2 * W)
                        mb3 = mbt[:, None, :].to_broadcast([W, 4, 2 * W])
                        nc.vector.scalar_tensor_tensor(
                            out3, in3, EXP_A * SCALE, mb3, mult, add)
                    else:
                        if "exp" not in skip:
                            nc.scalar.activation(pt[:], sp[:], Exp, scale=SCALE)
                        # causal mask on T1 blocks (cols 0, 256, 512, 768)
                        pt3 = pt[:].rearrange("p (g x) -> p g x", x=2 * W)
                        t1 = pt3[:, :, 0:W]
                        mb = mtile[:, None, :].to_broadcast([W, 4, W])
                        if "mask" not in skip:
                            nc.gpsimd.tensor_tensor(t1, t1, mb, mult)

                    pending.append((pt, pt_prev, vcs, oc, jl, p, c0))
                    pt_prev = pt
                    gp += 1
                    if "pv" not in skip and len(pending) > LAG:
                        emit_pv(pending.pop(0))

                if c == NCHUNK - 1 and "pv" not in skip:
                    while pending:
                        emit_pv(pending.pop(0))
    nc.finalize()
    return nc


def _prep_core_inputs(q2, k2, v2, core):
    s0 = core * BH_PER_CORE
    qTp = np.zeros((NPAIR, W, (WIN + 1) * W), BF16)
    kTp = np.zeros((NPAIR, W, T), BF16)
    for p in range(NPAIR):
        for h in range(2):
            bh = s0 + 2 * p + h
            qTp[p, h * 64:(h + 1) * 64, :T] = q2[bh].T.astype(BF16)
            kTp[p, h * 64:(h + 1) * 64, :] = k2[bh].T.astype(BF16)
    vr = v2[s0:s0 + BH_PER_CORE].reshape(
        BH_PER_CORE, WIN, W, D).transpose(0, 2, 1, 3)
    vp = np.zeros((BH_PER_CORE, W, WIN + 1, DV), BF16)
    vp[:, :, 1:, :D] = vr.astype(BF16)
    vp[:, :, :, D] = 1.0
    vp = np.ascontiguousarray(vp.reshape(BH_PER_CORE, W, (WIN + 1) * DV))
    m01 = (np.arange(W)[:, None] <= np.arange(W)[None, :])
    mask01 = m01.astype(BF16)
    maskB = np.empty((W, 2 * W), np.float32)
    maskB[:, :W] = np.where(m01, EXP_B, EXP_MASKED)
    maskB[:, W:] = EXP_B
    return {"qTp": qTp, "kT": kTp, "vp": vp, "mask01": mask01,
            "maskB": maskB}


def kernel(q, k, v, _trace=False):
    global last_perf
    from concourse.bass_utils import run_bass_kernel_spmd

    q = np.ascontiguousarray(np.asarray(q), dtype=np.float32)
    k = np.ascontiguousarray(np.asarray(k), dtype=np.float32)
    v = np.ascontiguousarray(np.asarray(v), dtype=np.float32)
    q2 = q.reshape(BH, T, D)
    k2 = k.reshape(BH, T, D)
    v2 = v.reshape(BH, T, D)

    if "nc" not in _nc_cache:
        _nc_cache["nc"] = _build_nc()
    nc = _nc_cache["nc"]

    in_maps = [_prep_core_inputs(q2, k2, v2, core) for core in range(NCORES)]
    res = run_bass_kernel_spmd(
        nc, in_maps, core_ids=list(range(NCORES)), trace=_trace)
    last_perf = res

    outs = []
    for core in range(NCORES):
        ot = np.asarray(res.results[core]["outT"]).astype(np.float32)
        # [4, W, WIN*DV] -> [4, W, WIN, DV] -> [4, WIN, W, DV] -> [4, T, DV]
        ot = ot.reshape(BH_PER_CORE, W, WIN, DV).transpose(0, 2, 1, 3)
        ot = ot.reshape(BH_PER_CORE, T, DV)
        o = ot[:, :, :D] / ot[:, :, D:DV]              # normalize
        outs.append(o)                                 # [4, T, 64]
    full = np.concatenate(outs, axis=0)                # [32, T, 64]
    return full.reshape(B, H, T, D)


# revision 47
# speedup vs baseline: 1.0387x; 1.0387x over previous
"""LocalAttention Bass kernel for Trainium2 (8 NeuronCores).

Problem: B=4 H=8 T=8192 D=64, window=128, look_backward=1, causal.
Sharding: pure (B*H) data parallelism — 32 heads / 8 cores = 4 heads each,
processed as 2 head-pairs so q/k DMAs use all 128 SBUF partitions.

Device algorithm (per head, per 128-token window w):
  S^T[k, q] = K_w' @ Q_w^T      (keys on partitions)
  P = exp(S^T * D^-0.5) * causal01
  O[q, 0:64] = P^T @ V, O[q, 64] = P^T @ 1   (row-sums ride as V's ones col)
Host divides O by the row-sum column and reshapes back.

All SBUF data is bf16 (PSUM stays fp32). Engine balance (measured ns):
  * S matmuls: K=64 per head -> heads A/B at tile_position (0,0)/(64,0)
    stream CONCURRENTLY (window-major emission, head-major PSUM banks).
  * PV matmuls: P blocks [128,128] STATIONARY (128 weight cols -> FWL,
    ~55ns/mm), [V|1] slots [128,65] moving -> out [queries, 65].
  * exp 64 pairs split: ~2/3 on ACT (exact, ~1330/pair) and ~1/3 on DVE
    via Schraudolph bit-trick (~1.8% rms; fused with the causal mask:
    int16(sp*184.665*SCALE + maskB) bitcast as bf16; masked lanes get
    bias 2048 -> tiny positive denormals, NEVER 0/negative-bias which
    would hit the bf16 NaN encodings).
  * ACT-path causal mask: mostly Pool tensor_tensor (~1.4us), some DVE
    tensor_tensor (~0.4us, 2x bf16 mode).
  * output cast PSUM->bf16: one DVE tensor_copy per pair (~0.5us).
  * PV emission lags S/exp by LAG pairs (software pipeline).
  * chunk loads split into two half-tiles, first-needed halves DMA'd
    first, so compute starts ~6us earlier (dependency tracking is
    tile-granular).

PSUM layout per 2-window pair j (windows w=2j, 2j+1), HEAD-major so the
two concurrent head streams hit different banks:
  sp [128, 1024] = [A(w) | A(w+1) | B(w) | B(w+1)], blocks = [T1 | T0next]
  op [128, 512]  = [A_w | A_w1 | B_w | B_w1], 128-col strides, 65 used.

Host-side shard prep (inside kernel(), numpy):
  qTp [2, 128, 8320]  — head-pair Q^T, one zero window appended
  kT  [2, 128, 8192]  — head-pair K^T
  vp  [4, 128, 4225]  — per head: 65 V slots x [V(64) | 1], slot 0 zeroed
  mask01 [128, 128]   — within-window causal 0/1 (k <= q), bf16
  maskB  [128, 256]   — f32 [16249*mask01 + 2048*(1-mask01) | 16249]
Output:
  outT [4, 128, 4160] — outT[h, q%W, w*65+i]: cols 0..63 unnormalized O,
                        col 64 row-sum r
"""

import numpy as np
import ml_dtypes

BF16 = np.dtype(ml_dtypes.bfloat16)

B, H, T, D = 4, 8, 8192, 64
W = 128                     # window size
WIN = T // W                # 64 windows per head
NCORES = 8
BH = B * H                  # 32
BH_PER_CORE = BH // NCORES  # 4
NPAIR = BH_PER_CORE // 2    # 2 head pairs per core
CHUNK_W = 16                # windows per load chunk
HC = CHUNK_W // 2           # half-chunk windows (8)
NCHUNK = WIN // CHUNK_W     # 4
NJ = CHUNK_W // 2           # 2-window pairs per chunk (8)
LAG = 2                     # software-pipeline depth in pairs
SCALE = float(D) ** -0.5
DV = D + 1                  # 65

# Schraudolph bf16-exp constants: bits = round(x*EXP_A + EXP_B)
EXP_A = 128.0 / float(np.log(2.0))   # 184.6650
EXP_B = 16256.0 - 7.0                # bias, C=7 centers the rel error
EXP_MASKED = 2048.0                  # masked-lane bias: tiny positive P

_nc_cache = {}
last_perf = None


def _build_nc(skip=()):
    import concourse.tile as tile
    from concourse import bacc
    from concourse import mybir
    from contextlib import ExitStack

    f32 = mybir.dt.float32
    bf16 = mybir.dt.bfloat16
    i16 = mybir.dt.int16
    Exp = mybir.ActivationFunctionType.Exp
    mult = mybir.AluOpType.mult
    add = mybir.AluOpType.add

    nc = bacc.Bacc()
    qTp = nc.dram_tensor("qTp", [NPAIR, W, (WIN + 1) * W], bf16,
                         kind="ExternalInput")
    kT = nc.dram_tensor("kT", [NPAIR, W, T], bf16, kind="ExternalInput")
    vp = nc.dram_tensor("vp", [BH_PER_CORE, W, (WIN + 1) * DV], bf16,
                        kind="ExternalInput")
    mask = nc.dram_tensor("mask01", [W, W], bf16, kind="ExternalInput")
    maskB = nc.dram_tensor("maskB", [W, 2 * W], f32, kind="ExternalInput")
    outT = nc.dram_tensor("outT", [BH_PER_CORE, W, WIN * DV], bf16,
                          kind="ExternalOutput")

    with tile.TileContext(nc) as tc, ExitStack() as ctx:
        cpool = ctx.enter_context(tc.tile_pool(name="cpool", bufs=1))
        qpool = ctx.enter_context(tc.tile_pool(name="qpool", bufs=3))
        kpool = ctx.enter_context(tc.tile_pool(name="kpool", bufs=3))
        vpool = ctx.enter_context(tc.tile_pool(name="vpool", bufs=6))
        opool = ctx.enter_context(tc.tile_pool(name="opool", bufs=3))
        ppool = ctx.enter_context(tc.tile_pool(name="ppool", bufs=5))
        spsum = ctx.enter_context(tc.tile_pool(name="spsum", bufs=3, space="PSUM"))
        opsum = ctx.enter_context(tc.tile_pool(name="opsum", bufs=2, space="PSUM"))

        mtile = cpool.tile([W, W], bf16)
        nc.sync.dma_start(mtile[:], mask[:])
        mbt = cpool.tile([W, 2 * W], f32)
        nc.sync.dma_start(mbt[:], maskB[:])
        z128 = cpool.tile([W, W], bf16)      # P for the all-masked pad window
        nc.vector.memset(z128[:], 0.0)

        mm = nc.tensor.matmul

        def emit_pv(item):
            """PV + output cast for one 2-window pair (P stationary).
            Emits the chunk's stores after its last pair is cast."""
            pt, pt_prev, vcs, oc, jl, p, c0 = item
            sbase = 0
            op = opsum.tile([W, 4 * W], f32, tag="op")
            for h in range(2):
                vc = vcs[h]
                for i in range(2):
                    wl = 2 * jl + i
                    oco = (2 * h + i) * W
                    s0 = (wl - sbase) * DV       # T0 slot offset in vc
                    # T0 contribution: keys of window wl-1
                    if i == 1:
                        t0 = pt[:, 2 * h * 256 + W:2 * h * 256 + 2 * W]
                    elif pt_prev is not None:
                        t0 = pt_prev[:, (2 * h + 1) * 256 + W:
                                     (2 * h + 1) * 256 + 2 * W]
                    else:
                        t0 = z128[:]
                    mm(op[:, oco:oco + DV], t0, vc[:, s0:s0 + DV],
                       start=True, stop=False)
                    # T1 contribution: keys of window wl
                    mm(op[:, oco:oco + DV],
                       pt[:, (2 * h + i) * 256:(2 * h + i) * 256 + W],
                       vc[:, s0 + DV:s0 + 2 * DV],
                       start=False, stop=True)
            if "ocopy" not in skip:
                # one cast for both heads: [128, (h,i,65)] views
                src = op[:].rearrange("p (h i x) -> p h i x", h=2, x=W)
                src = src[:, :, :, 0:DV]
                dst = oc[:].rearrange("p (h j x) -> p h j x", h=2, x=2 * DV)
                dst = dst[:, :, jl, :].rearrange("p h (i e) -> p h i e", e=DV)
                nc.vector.tensor_copy(dst, src)
            if jl == NJ - 1 and "store" not in skip:
                o0 = (c0 // W) * DV
                for h in range(2):
                    nc.sync.dma_start(
                        outT[2 * p + h, :, o0:o0 + CHUNK_W * DV],
                        oc[:, h * NJ * 2 * DV:(h + 1) * NJ * 2 * DV])

        gp = 0  # global pair counter
        for p in range(NPAIR):
            pt_prev = None
            pending = []
            for c in range(NCHUNK):
                c0 = c * CHUNK_W * W
                qc = qpool.tile([W, (CHUNK_W + 1) * W], bf16, tag="qc")
                kc = kpool.tile([W, CHUNK_W * W], bf16, tag="kc")
                vc0 = vpool.tile([W, (CHUNK_W + 1) * DV], bf16, tag="vc")
                vc1 = vpool.tile([W, (CHUNK_W + 1) * DV], bf16, tag="vc")
                vcs = [vc0, vc1]
                if "loads" not in skip:
                    # cold start: issue q/k from the idle Activation and
                    # Pool queues so the three loads start ~simultaneously
                    # (consecutive Sync dma_starts issue ~0.7us apart);
                    # only gpsimd/SP/Activation can initiate DMAs
                    cold = p == 0 and c == 0
                    eq = nc.scalar if cold else nc.sync
                    ek = nc.gpsimd if cold else nc.sync
                    eq.dma_start(qc[:], qTp[p, :, c0:c0 + (CHUNK_W + 1) * W])
                    ek.dma_start(kc[:], kT[p, :, c0:c0 + CHUNK_W * W])
                    for h in range(2):
                        v0 = c * CHUNK_W * DV
                        nc.sync.dma_start(
                            vcs[h][:],
                            vp[2 * p + h, :, v0:v0 + (CHUNK_W + 1) * DV])
                oc = opool.tile([W, 2 * NJ * 2 * DV], bf16, tag="oc")

                for jl in range(NJ):
                    w0 = 2 * jl
                    qb = 0
                    sp = spsum.tile([W, 4 * 2 * W], f32, tag="sp")
                    if "smm" not in skip:
                        for i in range(2):
                            wl = w0 + i
                            for h in range(2):
                                hb = h * 64
                                mm(sp[:, (2 * h + i) * 256:(2 * h + i + 1) * 256],
                                   kc[hb:hb + 64,
                                      (wl - qb) * W:(wl - qb + 1) * W],
                                   qc[hb:hb + 64,
                                      (wl - qb) * W:(wl - qb + 2) * W],
                                   start=True, stop=True)

                    pt = ppool.tile([W, 4 * 2 * W], bf16, tag="pt")
                    if gp % 3 == 1 and "dve_exp" not in skip:
                        # fused Schraudolph exp + causal mask on DVE
                        pt_i = pt[:].bitcast(i16)
                        out3 = pt_i.rearrange("p (g x) -> p g x", x=2 * W)
                        in3 = sp[:].rearrange("p (g x) -> p g x", x=2 * W)
                        mb3 = mbt[:, None, :].to_broadcast([W, 4, 2 * W])
                        nc.vector.scalar_tensor_tensor(
                            out3, in3, EXP_A * SCALE, mb3, mult, add)
                    else:
                        if "exp" not in skip:
                            nc.scalar.activation(pt[:], sp[:], Exp, scale=SCALE)
                        # causal mask on T1 blocks (cols 0, 256, 512, 768)
                        pt3 = pt[:].rearrange("p (g x) -> p g x", x=2 * W)
                        t1 = pt3[:, :, 0:W]
                        mb = mtile[:, None, :].to_broadcast([W, 4, W])
                        if "mask" not in skip:
                            nc.gpsimd.tensor_tensor(t1, t1, mb, mult)

                    pending.append((pt, pt_prev, vcs, oc, jl, p, c0))
                    pt_prev = pt
                    gp += 1
                    if "pv" not in skip and len(pending) > LAG:
                        emit_pv(pending.pop(0))

                if c == NCHUNK - 1 and "pv" not in skip:
                    while pending:
                        emit_pv(pending.pop(0))
    nc.finalize()
    return nc


def _prep_core_inputs(q2, k2, v2, core):
    s0 = core * BH_PER_CORE
    qTp = np.zeros((NPAIR, W, (WIN + 1) * W), BF16)
    kTp = np.zeros((NPAIR, W, T), BF16)
    for p in range(NPAIR):
        for h in range(2):
            bh = s0 + 2 * p + h
            qTp[p, h * 64:(h + 1) * 64, :T] = q2[bh].T.astype(BF16)
            kTp[p, h * 64:(h + 1) * 64, :] = k2[bh].T.astype(BF16)
    vr = v2[s0:s0 + BH_PER_CORE].reshape(
        BH_PER_CORE, WIN, W, D).transpose(0, 2, 1, 3)
    vp = np.zeros((BH_PER_CORE, W, WIN + 1, DV), BF16)
    vp[:, :, 1:, :D] = vr.astype(BF16)
    vp[:, :, :, D] = 1.0
    vp = np.ascontiguousarray(vp.reshape(BH_PER_CORE, W, (WIN + 1) * DV))
    m01 = (np.arange(W)[:, None] <= np.arange(W)[None, :])
    mask01 = m01.astype(BF16)
    maskB = np.empty((W, 2 * W), np.float32)
    maskB[:, :W] = np.where(m01, EXP_B, EXP_MASKED)
    maskB[:, W:] = EXP_B
    return {"qTp": qTp, "kT": kTp, "vp": vp, "mask01": mask01,
            "maskB": maskB}


def kernel(q, k, v, _trace=False):
    global last_perf
    from concourse.bass_utils import run_bass_kernel_spmd

    q = np.ascontiguousarray(np.asarray(q), dtype=np.float32)
    k = np.ascontiguousarray(np.asarray(k), dtype=np.float32)
    v = np.ascontiguousarray(np.asarray(v), dtype=np.float32)
    q2 = q.reshape(BH, T, D)
    k2 = k.reshape(BH, T, D)
    v2 = v.reshape(BH, T, D)

    if "nc" not in _nc_cache:
        _nc_cache["nc"] = _build_nc()
    nc = _nc_cache["nc"]

    in_maps = [_prep_core_inputs(q2, k2, v2, core) for core in range(NCORES)]
    res = run_bass_kernel_spmd(
        nc, in_maps, core_ids=list(range(NCORES)), trace=_trace)
    last_perf = res

    outs = []
    for core in range(NCORES):
        ot = np.asarray(res.results[core]["outT"]).astype(np.float32)
        # [4, W, WIN*DV] -> [4, W, WIN, DV] -> [4, WIN, W, DV] -> [4, T, DV]
        ot = ot.reshape(BH_PER_CORE, W, WIN, DV).transpose(0, 2, 1, 3)
        ot = ot.reshape(BH_PER_CORE, T, DV)
        o = ot[:, :, :D] / ot[:, :, D:DV]              # normalize
        outs.append(o)                                 # [4, T, 64]
    full = np.concatenate(outs, axis=0)                # [32, T, 64]
    return full.reshape(B, H, T, D)


# revision 49
# speedup vs baseline: 1.0759x; 1.0358x over previous
"""LocalAttention Bass kernel for Trainium2 (8 NeuronCores).

Problem: B=4 H=8 T=8192 D=64, window=128, look_backward=1, causal.
Sharding: pure (B*H) data parallelism — 32 heads / 8 cores = 4 heads each,
processed as 2 head-pairs so q/k DMAs use all 128 SBUF partitions.

Device algorithm (per head, per 128-token window w):
  S^T[k, q] = K_w' @ Q_w^T      (keys on partitions)
  P = exp(S^T * D^-0.5) * causal01
  O[q, 0:64] = P^T @ V, O[q, 64] = P^T @ 1   (row-sums ride as V's ones col)
Host divides O by the row-sum column and reshapes back.

All SBUF data is bf16 (PSUM stays fp32). Engine balance (measured ns):
  * S matmuls: K=64 per head -> heads A/B at tile_position (0,0)/(64,0)
    stream CONCURRENTLY (window-major emission, head-major PSUM banks).
  * PV matmuls: P blocks [128,128] STATIONARY (128 weight cols -> FWL,
    ~55ns/mm), [V|1] slots [128,65] moving -> out [queries, 65].
  * exp 64 pairs split: ~2/3 on ACT (exact, ~1330/pair) and ~1/3 on DVE
    via Schraudolph bit-trick (~1.8% rms; fused with the causal mask:
    int16(sp*184.665*SCALE + maskB) bitcast as bf16; masked lanes get
    bias 2048 -> tiny positive denormals, NEVER 0/negative-bias which
    would hit the bf16 NaN encodings).
  * ACT-path causal mask: mostly Pool tensor_tensor (~1.4us), some DVE
    tensor_tensor (~0.4us, 2x bf16 mode).
  * output cast PSUM->bf16: one DVE tensor_copy per pair (~0.5us).
  * PV emission lags S/exp by LAG pairs (software pipeline).
  * chunk loads split into two half-tiles, first-needed halves DMA'd
    first, so compute starts ~6us earlier (dependency tracking is
    tile-granular).

PSUM layout per 2-window pair j (windows w=2j, 2j+1), HEAD-major so the
two concurrent head streams hit different banks:
  sp [128, 1024] = [A(w) | A(w+1) | B(w) | B(w+1)], blocks = [T1 | T0next]
  op [128, 512]  = [A_w | A_w1 | B_w | B_w1], 128-col strides, 65 used.

Host-side shard prep (inside kernel(), numpy):
  qTp [2, 128, 8320]  — head-pair Q^T, one zero window appended
  kT  [2, 128, 8192]  — head-pair K^T
  vp  [4, 128, 4225]  — per head: 65 V slots x [V(64) | 1], slot 0 zeroed
  mask01 [128, 128]   — within-window causal 0/1 (k <= q), bf16
  maskB  [128, 256]   — f32 [16249*mask01 + 2048*(1-mask01) | 16249]
Output:
  outT [4, 128, 4160] — outT[h, q%W, w*65+i]: cols 0..63 unnormalized O,
                        col 64 row-sum r
"""

import numpy as np
import ml_dtypes

BF16 = np.dtype(ml_dtypes.bfloat16)

B, H, T, D = 4, 8, 8192, 64
W = 128                     # window size
WIN = T // W                # 64 windows per head
NCORES = 8
BH = B * H                  # 32
BH_PER_CORE = BH // NCORES  # 4
NPAIR = BH_PER_CORE // 2    # 2 head pairs per core
CHUNK_W = 16                # windows per load chunk
HC = CHUNK_W // 2           # half-chunk windows (8)
NCHUNK = WIN // CHUNK_W     # 4
NJ = CHUNK_W // 2           # 2-window pairs per chunk (8)
LAG = 2                     # software-pipeline depth in pairs
SCALE = float(D) ** -0.5
DV = D + 1                  # 65

# Schraudolph bf16-exp constants: bits = round(x*EXP_A + EXP_B)
EXP_A = 128.0 / float(np.log(2.0))   # 184.6650
EXP_B = 16256.0 - 7.0                # bias, C=7 centers the rel error
EXP_MASKED = 2048.0                  # masked-lane bias: tiny positive P

_nc_cache = {}
last_perf = None


def _build_nc(skip=()):
    import concourse.tile as tile
    from concourse import bacc
    from concourse import mybir
    from contextlib import ExitStack

    f32 = mybir.dt.float32
    bf16 = mybir.dt.bfloat16
    i16 = mybir.dt.int16
    Exp = mybir.ActivationFunctionType.Exp
    mult = mybir.AluOpType.mult
    add = mybir.AluOpType.add

    nc = bacc.Bacc()
    qTp = nc.dram_tensor("qTp", [NPAIR, W, (WIN + 1) * W], bf16,
                         kind="ExternalInput")
    kT = nc.dram_tensor("kT", [NPAIR, W, T], bf16, kind="ExternalInput")
    vp = nc.dram_tensor("vp", [BH_PER_CORE, W, (WIN + 1) * DV], bf16,
                        kind="ExternalInput")
    mask = nc.dram_tensor("mask01", [W, W], bf16, kind="ExternalInput")
    maskB = nc.dram_tensor("maskB", [W, 2 * W], f32, kind="ExternalInput")
    outT = nc.dram_tensor("outT", [BH_PER_CORE, W, WIN * DV], bf16,
                          kind="ExternalOutput")

    with tile.TileContext(nc) as tc, ExitStack() as ctx:
        cpool = ctx.enter_context(tc.tile_pool(name="cpool", bufs=1))
        qpool = ctx.enter_context(tc.tile_pool(name="qpool", bufs=3))
        kpool = ctx.enter_context(tc.tile_pool(name="kpool", bufs=3))
        vpool = ctx.enter_context(tc.tile_pool(name="vpool", bufs=6))
        opool = ctx.enter_context(tc.tile_pool(name="opool", bufs=4))
        ppool = ctx.enter_context(tc.tile_pool(name="ppool", bufs=6))
        spsum = ctx.enter_context(tc.tile_pool(name="spsum", bufs=3, space="PSUM"))
        opsum = ctx.enter_context(tc.tile_pool(name="opsum", bufs=2, space="PSUM"))

        mtile = cpool.tile([W, W], bf16)
        nc.sync.dma_start(mtile[:], mask[:])
        mbt = cpool.tile([W, 2 * W], f32)
        nc.sync.dma_start(mbt[:], maskB[:])
        z128 = cpool.tile([W, W], bf16)      # P for the all-masked pad window
        nc.vector.memset(z128[:], 0.0)

        mm = nc.tensor.matmul

        def emit_pv(item):
            """PV + output cast for one 2-window pair (P stationary).
            Emits the chunk's stores after its last pair is cast."""
            pt, pt_prev, vcs, oc, jl, p, c0 = item
            sbase = 0
            op = opsum.tile([W, 4 * W], f32, tag="op")
            for h in range(2):
                vc = vcs[h]
                for i in range(2):
                    wl = 2 * jl + i
                    oco = (2 * h + i) * W
                    s0 = (wl - sbase) * DV       # T0 slot offset in vc
                    # T0 contribution: keys of window wl-1
                    if i == 1:
                        t0 = pt[:, 2 * h * 256 + W:2 * h * 256 + 2 * W]
                    elif pt_prev is not None:
                        t0 = pt_prev[:, (2 * h + 1) * 256 + W:
                                     (2 * h + 1) * 256 + 2 * W]
                    else:
                        t0 = z128[:]
                    mm(op[:, oco:oco + DV], t0, vc[:, s0:s0 + DV],
                       start=True, stop=False)
                    # T1 contribution: keys of window wl
                    mm(op[:, oco:oco + DV],
                       pt[:, (2 * h + i) * 256:(2 * h + i) * 256 + W],
                       vc[:, s0 + DV:s0 + 2 * DV],
                       start=False, stop=True)
            if "ocopy" not in skip:
                # one cast for both heads: [128, (h,i,65)] views
                src = op[:].rearrange("p (h i x) -> p h i x", h=2, x=W)
                src = src[:, :, :, 0:DV]
                dst = oc[:].rearrange("p (h j x) -> p h j x", h=2, x=2 * DV)
                dst = dst[:, :, jl, :].rearrange("p h (i e) -> p h i e", e=DV)
                nc.vector.tensor_copy(dst, src)
            if jl == NJ - 1 and "store" not in skip:
                o0 = (c0 // W) * DV
                for h in range(2):
                    nc.sync.dma_start(
                        outT[2 * p + h, :, o0:o0 + CHUNK_W * DV],
                        oc[:, h * NJ * 2 * DV:(h + 1) * NJ * 2 * DV])

        gp = 0  # global pair counter
        for p in range(NPAIR):
            pt_prev = None
            pending = []
            for c in range(NCHUNK):
                c0 = c * CHUNK_W * W
                qc = qpool.tile([W, (CHUNK_W + 1) * W], bf16, tag="qc")
                kc = kpool.tile([W, CHUNK_W * W], bf16, tag="kc")
                vc0 = vpool.tile([W, (CHUNK_W + 1) * DV], bf16, tag="vc")
                vc1 = vpool.tile([W, (CHUNK_W + 1) * DV], bf16, tag="vc")
                vcs = [vc0, vc1]
                if "loads" not in skip:
                    nc.sync.dma_start(qc[:], qTp[p, :, c0:c0 + (CHUNK_W + 1) * W])
                    nc.sync.dma_start(kc[:], kT[p, :, c0:c0 + CHUNK_W * W])
                    for h in range(2):
                        v0 = c * CHUNK_W * DV
                        nc.sync.dma_start(
                            vcs[h][:],
                            vp[2 * p + h, :, v0:v0 + (CHUNK_W + 1) * DV])
                oc = opool.tile([W, 2 * NJ * 2 * DV], bf16, tag="oc")

                for jl in range(NJ):
                    w0 = 2 * jl
                    qb = 0
                    sp = spsum.tile([W, 4 * 2 * W], f32, tag="sp")
                    if "smm" not in skip:
                        for i in range(2):
                            wl = w0 + i
                            for h in range(2):
                                hb = h * 64
                                mm(sp[:, (2 * h + i) * 256:(2 * h + i + 1) * 256],
                                   kc[hb:hb + 64,
                                      (wl - qb) * W:(wl - qb + 1) * W],
                                   qc[hb:hb + 64,
                                      (wl - qb) * W:(wl - qb + 2) * W],
                                   start=True, stop=True)

                    pt = ppool.tile([W, 4 * 2 * W], bf16, tag="pt")
                    if gp % 3 == 1 and "dve_exp" not in skip:
                        # fused Schraudolph exp + causal mask on DVE
                        pt_i = pt[:].bitcast(i16)
                        out3 = pt_i.rearrange("p (g x) -> p g x", x=2 * W)
                        in3 = sp[:].rearrange("p (g x) -> p g x", x=2 * W)
                        mb3 = mbt[:, None, :].to_broadcast([W, 4, 2 * W])
                        nc.vector.scalar_tensor_tensor(
                            out3, in3, EXP_A * SCALE, mb3, mult, add)
                    else:
                        if "exp" not in skip:
                            nc.scalar.activation(pt[:], sp[:], Exp, scale=SCALE)
                        # causal mask on T1 blocks (cols 0, 256, 512, 768)
                        pt3 = pt[:].rearrange("p (g x) -> p g x", x=2 * W)
                        t1 = pt3[:, :, 0:W]
                        mb = mtile[:, None, :].to_broadcast([W, 4, W])
                        if "mask" not in skip:
                            nc.gpsimd.tensor_tensor(t1, t1, mb, mult)

                    pending.append((pt, pt_prev, vcs, oc, jl, p, c0))
                    pt_prev = pt
                    gp += 1
                    if "pv" not in skip and len(pending) > LAG:
                        emit_pv(pending.pop(0))

                if c == NCHUNK - 1 and "pv" not in skip:
                    while pending:
                        emit_pv(pending.pop(0))
    nc.finalize()
    return nc


def _prep_core_inputs(q2, k2, v2, core):
    s0 = core * BH_PER_CORE
    qTp = np.zeros((NPAIR, W, (WIN + 1) * W), BF16)
    kTp = np.zeros((NPAIR, W, T), BF16)
    for p in range(NPAIR):
        for h in range(2):
            bh = s0 + 2 * p + h
            qTp[p, h * 64:(h + 1) * 64, :T] = q2[bh].T.astype(BF16)
            kTp[p, h * 64:(h + 1) * 64, :] = k2[bh].T.astype(BF16)
    vr = v2[s0:s0 + BH_PER_CORE].reshape(
        BH_PER_CORE, WIN, W, D).transpose(0, 2, 1, 3)
    vp = np.zeros((BH_PER_CORE, W, WIN + 1, DV), BF16)
    vp[:, :, 1:, :D] = vr.astype(BF16)
    vp[:, :, :, D] = 1.0
    vp = np.ascontiguousarray(vp.reshape(BH_PER_CORE, W, (WIN + 1) * DV))
    m01 = (np.arange(W)[:, None] <= np.arange(W)[None, :])
    mask01 = m01.astype(BF16)
    maskB = np.empty((W, 2 * W), np.float32)
    maskB[:, :W] = np.where(m01, EXP_B, EXP_MASKED)
    maskB[:, W:] = EXP_B
    return {"qTp": qTp, "kT": kTp, "vp": vp, "mask01": mask01,
            "maskB": maskB}


def kernel(q, k, v, _trace=False):
    global last_perf
    from concourse.bass_utils import run_bass_kernel_spmd

    q = np.ascontiguousarray(np.asarray(q), dtype=np.float32)
    k = np.ascontiguousarray(np.asarray(k), dtype=np.float32)
    v = np.ascontiguousarray(np.asarray(v), dtype=np.float32)
    q2 = q.reshape(BH, T, D)
    k2 = k.reshape(BH, T, D)
    v2 = v.reshape(BH, T, D)

    if "nc" not in _nc_cache:
        _nc_cache["nc"] = _build_nc()
    nc = _nc_cache["nc"]

    in_maps = [_prep_core_inputs(q2, k2, v2, core) for core in range(NCORES)]
    res = run_bass_kernel_spmd(
        nc, in_maps, core_ids=list(range(NCORES)), trace=_trace)
    last_perf = res

    outs = []
    for core in range(NCORES):
        ot = np.asarray(res.results[core]["outT"]).astype(np.float32)
        # [4, W, WIN*DV] -> [4, W, WIN, DV] -> [4, WIN, W, DV] -> [4, T, DV]
        ot = ot.reshape(BH_PER_CORE, W, WIN, DV).transpose(0, 2, 1, 3)
        ot = ot.reshape(BH_PER_CORE, T, DV)
        o = ot[:, :, :D] / ot[:, :, D:DV]              # normalize
        outs.append(o)                                 # [4, T, 64]
    full = np.concatenate(outs, axis=0)                # [32, T, 64]
    return full.reshape(B, H, T, D)
